# revision 14
# baseline (speedup 1.0000x reference)
"""Mexican-hat wavelet KAN layer + BatchNorm (training stats) on 8 TRN2 cores.

Reference computation (B=I=O=512):
    t   = (x[b,i] - bias[i,o]) / scale[i,o]
    wav = NORM * (t^2 - 1) * exp(-t^2/2)
    y   = einsum('bio,io->bo', wav, weight)
    out = batchnorm_train(y, gamma, beta)          # biased stats over batch

Sharding: output-feature parallel.  Each of the 8 cores computes the FULL
batch for a 64-wide slice of O.  BatchNorm stats are over the batch dim,
which is fully local per core -> no collectives at all.

Fast path (used when scale/bias are constant along O, which holds for the
canonical inputs where scale==1, bias==0): the wavelet then depends only on
(b,i), so the cubic (B,I,O) tensor collapses to a (B,I) wavelet followed by
a plain matmul with weight.  A general fallback path evaluates the full
per-(i,o) wavelet on device when the structure check fails.

The kernel is written in raw Bass (explicit semaphores, standalone wait_ge
instructions) because this walrus codegen caps every instruction at ONE
sync-wait: Tile's auto-semaphores attach multiple waits to one instruction
and fail to compile.
"""

import math

import numpy as np

import concourse.bass as bass
from concourse import mybir
from concourse.bass_utils import run_bass_kernel_spmd

B, I, O = 512, 512, 512
N_CORES = 8
OS = O // N_CORES          # 64 output features per core
KP = 128                   # partition chunk of the contraction dim
NK = I // KP               # 4 chunks
MEXHAT_NORM = 2.0 / (math.sqrt(3.0) * math.pi**0.25)
BN_EPS = 1e-5
FP32 = mybir.dt.float32
F = mybir.ActivationFunctionType
A = mybir.AluOpType

WCOLS = NK * OS + 2        # packed weight cols + gamma + beta
XCOLS_F = 2 + B            # fast path per-chunk: [1/s | -b/s | x^T]
AB_F = NK * XCOLS_F + WCOLS          # fast-path packed input width
AB_G = NK * B + WCOLS + 2 * NK * OS  # general-path packed input width
G_XT0 = 0                  # general-path column offsets
G_WC0 = NK * B
G_IV0 = G_WC0 + WCOLS
G_NB0 = G_IV0 + NK * OS

_programs: dict[str, bass.Bass] = {}


def _build_fast() -> bass.Bass:
    nc = bass.Bass("TRN2", target_bir_lowering=False, debug=False,
                   num_devices=N_CORES)
    ab = nc.dram_tensor("ab", [KP, AB_F], FP32, kind="ExternalInput").ap()
    yT = nc.dram_tensor("yT", [OS, B], FP32, kind="ExternalOutput").ap()

    wcb = nc.alloc_sbuf_tensor("wcb", [KP, WCOLS], FP32).ap()
    xa = [nc.alloc_sbuf_tensor(f"xa{k}", [KP, XCOLS_F], FP32).ap()
          for k in range(NK)]
    u = [nc.alloc_sbuf_tensor(f"u{k}", [KP, B], FP32).ap() for k in range(NK)]
    e = [nc.alloc_sbuf_tensor(f"e{k}", [KP, B], FP32).ap() for k in range(NK)]
    wav = [nc.alloc_sbuf_tensor(f"wav{k}", [KP, B], FP32).ap()
           for k in range(NK)]
    psum = nc.alloc_psum_tensor("psum", [OS, B], FP32).ap()
    ysb = nc.alloc_sbuf_tensor("ysb", [OS, B], FP32).ap()
    sq = nc.alloc_sbuf_tensor("sqb", [OS, B], FP32).ap()
    out_sb = nc.alloc_sbuf_tensor("out_sb", [OS, B], FP32).ap()
    ysum = nc.alloc_sbuf_tensor("ysum", [OS, 1], FP32).ap()
    ssq = nc.alloc_sbuf_tensor("ssq", [OS, 1], FP32).ap()
    mean = nc.alloc_sbuf_tensor("mean", [OS, 1], FP32).ap()
    msq = nc.alloc_sbuf_tensor("msq", [OS, 1], FP32).ap()
    m2 = nc.alloc_sbuf_tensor("m2", [OS, 1], FP32).ap()
    var = nc.alloc_sbuf_tensor("var", [OS, 1], FP32).ap()
    std = nc.alloc_sbuf_tensor("std", [OS, 1], FP32).ap()
    rstd = nc.alloc_sbuf_tensor("rstd", [OS, 1], FP32).ap()
    ga = nc.alloc_sbuf_tensor("ga", [OS, 1], FP32).ap()
    mga = nc.alloc_sbuf_tensor("mga", [OS, 1], FP32).ap()
    bb = nc.alloc_sbuf_tensor("bb", [OS, 1], FP32).ap()

    gamma_ap = wcb[0:OS, NK * OS:NK * OS + 1]
    beta_ap = wcb[0:OS, NK * OS + 1:NK * OS + 2]

    with nc.Block() as block, \
         nc.semaphore("sw") as sw, \
         nc.semaphore("sx0") as sx0, \
         nc.semaphore("sx1") as sx1, \
         nc.semaphore("sx2") as sx2, \
         nc.semaphore("sx3") as sx3, \
         nc.semaphore("sa") as sa, \
         nc.semaphore("sv") as sv, \
         nc.semaphore("spe") as spe, \
         nc.semaphore("so") as so:
        sx = [sx0, sx1, sx2, sx3]

        @block.sync
        def _(sp):
            sp.dma_start(out=wcb[:], in_=ab[:, NK * XCOLS_F:]).then_inc(sw, 16)
            for k in range(NK):
                sp.dma_start(
                    out=xa[k][:],
                    in_=ab[:, k * XCOLS_F:(k + 1) * XCOLS_F]).then_inc(sx[k], 16)
            sp.wait_ge(sv, 13)
            sp.dma_start(out=yT[:], in_=out_sb[:]).then_inc(so, 16)
            sp.wait_ge(so, 16)

        @block.scalar
        def _(act):
            for k in range(NK):
                act.wait_ge(sx[k], 16)
                # u = ((x - b)/s)^2 ; the ACT free affine does the normalize
                act.activation(u[k][:], xa[k][:, 2:], F.Square,
                               bias=xa[k][:, 1:2],
                               scale=xa[k][:, 0:1]).then_inc(sa)
                # self-wait: ACT pipeline doesn't interlock same-engine RAW
                act.wait_ge(sa, 2 * k + 1)
                # e = exp(-u/2); MEXHAT_NORM is folded into the weights
                act.activation(e[k][:], u[k][:], F.Exp, bias=0.0,
                               scale=-0.5).then_inc(sa)
            # BN tail: evacuate PSUM once, fusing sum(y) into the copy
            act.wait_ge(spe, 1)
            act.activation(ysb[:], psum[:], F.Copy, bias=0.0, scale=1.0,
                           accum_out=ysum[:]).then_inc(sa)        # sa=9
            act.wait_ge(sa, 9)
            act.activation(sq[:], ysb[:], F.Square, bias=0.0, scale=1.0,
                           accum_out=ssq[:]).then_inc(sa)         # sa=10
            act.wait_ge(sv, 8)
            act.activation(std[:], var[:], F.Sqrt, bias=0.0,
                           scale=1.0).then_inc(sa)                # sa=11

        @block.vector
        def _(dve):
            for k in range(NK):
                dve.wait_ge(sa, 2 * k + 2)
                # wav = (u - 1) * e
                dve.scalar_tensor_tensor(out=wav[k][:], in0=u[k][:], scalar=1.0,
                                         in1=e[k][:], op0=A.subtract,
                                         op1=A.mult).then_inc(sv)  # sv=k+1
            dve.wait_ge(sa, 9)
            dve.tensor_scalar_mul(mean[:], ysum[:], 1.0 / B).then_inc(sv)  # 5
            dve.wait_ge(sa, 10)
            dve.tensor_scalar(out=msq[:], in0=ssq[:], scalar1=1.0 / B,
                              scalar2=BN_EPS, op0=A.mult,
                              op1=A.add).then_inc(sv)              # 6
            dve.wait_ge(sv, 5)
            dve.tensor_mul(m2[:], mean[:], mean[:]).then_inc(sv)   # 7
            dve.wait_ge(sv, 7)
            dve.tensor_sub(var[:], msq[:], m2[:]).then_inc(sv)     # 8
            dve.wait_ge(sa, 11)
            dve.reciprocal(rstd[:], std[:]).then_inc(sv)           # 9
            dve.wait_ge(sw, 16)
            dve.wait_ge(sv, 9)
            dve.tensor_mul(ga[:], rstd[:], gamma_ap).then_inc(sv)  # 10
            dve.wait_ge(sv, 10)
            dve.tensor_mul(mga[:], mean[:], ga[:]).then_inc(sv)    # 11
            dve.wait_ge(sv, 11)
            dve.tensor_sub(bb[:], beta_ap, mga[:]).then_inc(sv)    # 12
            dve.wait_ge(sv, 12)
            dve.tensor_scalar(out=out_sb[:], in0=ysb[:], scalar1=ga[:],
                              scalar2=bb[:], op0=A.mult,
                              op1=A.add).then_inc(sv)              # 13

        @block.tensor
        def _(pe):
            pe.wait_ge(sw, 16)
            for k in range(NK):
                pe.wait_ge(sv, k + 1)
                mm = pe.matmul(psum[:], lhsT=wcb[:, k * OS:(k + 1) * OS],
                               rhs=wav[k][:], start=(k == 0),
                               stop=(k == NK - 1))
                if k == NK - 1:
                    mm.then_inc(spe)
    return nc


def _build_general() -> bass.Bass:
    """Full per-(i,o) wavelet: scale/bias vary along O.  ~64x the compute of
    the fast path; correctness fallback only."""
    nc = bass.Bass("TRN2", target_bir_lowering=False, debug=False,
                   num_devices=N_CORES)
    ab = nc.dram_tensor("ab", [KP, AB_G], FP32, kind="ExternalInput").ap()
    yT = nc.dram_tensor("yT", [OS, B], FP32, kind="ExternalOutput").ap()

    big = nc.alloc_sbuf_tensor("big", [KP, AB_G], FP32).ap()
    u = [nc.alloc_sbuf_tensor(f"u{j}", [KP, B], FP32).ap() for j in range(2)]
    e = [nc.alloc_sbuf_tensor(f"e{j}", [KP, B], FP32).ap() for j in range(2)]
    wv = [nc.alloc_sbuf_tensor(f"wv{j}", [KP, B], FP32).ap() for j in range(2)]
    psum = nc.alloc_psum_tensor("psum", [OS, B], FP32).ap()
    ysb = nc.alloc_sbuf_tensor("ysb", [OS, B], FP32).ap()
    sq = nc.alloc_sbuf_tensor("sqb", [OS, B], FP32).ap()
    out_sb = nc.alloc_sbuf_tensor("out_sb", [OS, B], FP32).ap()
    ysum = nc.alloc_sbuf_tensor("ysum", [OS, 1], FP32).ap()
    ssq = nc.alloc_sbuf_tensor("ssq", [OS, 1], FP32).ap()
    mean = nc.alloc_sbuf_tensor("mean", [OS, 1], FP32).ap()
    msq = nc.alloc_sbuf_tensor("msq", [OS, 1], FP32).ap()
    m2 = nc.alloc_sbuf_tensor("m2", [OS, 1], FP32).ap()
    var = nc.alloc_sbuf_tensor("var", [OS, 1], FP32).ap()
    std = nc.alloc_sbuf_tensor("std", [OS, 1], FP32).ap()
    rstd = nc.alloc_sbuf_tensor("rstd", [OS, 1], FP32).ap()
    ga = nc.alloc_sbuf_tensor("ga", [OS, 1], FP32).ap()
    mga = nc.alloc_sbuf_tensor("mga", [OS, 1], FP32).ap()
    bb = nc.alloc_sbuf_tensor("bb", [OS, 1], FP32).ap()

    gamma_ap = big[0:OS, G_WC0 + NK * OS:G_WC0 + NK * OS + 1]
    beta_ap = big[0:OS, G_WC0 + NK * OS + 1:G_WC0 + NK * OS + 2]
    NIT = OS * NK  # 256 (o, k) iterations

    with nc.Block() as block, \
         nc.semaphore("sin") as sin, \
         nc.semaphore("sa") as sa, \
         nc.semaphore("sv") as sv, \
         nc.semaphore("spe") as spe, \
         nc.semaphore("so") as so:

        @block.sync
        def _(sp):
            sp.dma_start(out=big[:], in_=ab[:]).then_inc(sin, 16)
            sp.wait_ge(sv, NIT + 9)
            sp.dma_start(out=yT[:], in_=out_sb[:]).then_inc(so, 16)
            sp.wait_ge(so, 16)

        @block.scalar
        def _(act):
            act.wait_ge(sin, 16)
            n = 0
            for o in range(OS):
                for k in range(NK):
                    col = k * OS + o
                    j = n % 2
                    if n >= 2:
                        # u[j]/e[j] were read by DVE stt #(n-2) -> sv >= n-1
                        act.wait_ge(sv, n - 1)
                    act.activation(
                        u[j][:], big[:, k * B:(k + 1) * B], F.Square,
                        bias=big[:, G_NB0 + col:G_NB0 + col + 1],
                        scale=big[:, G_IV0 + col:G_IV0 + col + 1]).then_inc(sa)
                    act.wait_ge(sa, 2 * n + 1)
                    act.activation(e[j][:], u[j][:], F.Exp, bias=0.0,
                                   scale=-0.5).then_inc(sa)
                    n += 1
            act.wait_ge(spe, NIT)
            act.activation(ysb[:], psum[:], F.Copy, bias=0.0, scale=1.0,
                           accum_out=ysum[:]).then_inc(sa)
            act.wait_ge(sa, 2 * NIT + 1)
            act.activation(sq[:], ysb[:], F.Square, bias=0.0, scale=1.0,
                           accum_out=ssq[:]).then_inc(sa)
            act.wait_ge(sv, NIT + 4)
            act.activation(std[:], var[:], F.Sqrt, bias=0.0,
                           scale=1.0).then_inc(sa)

        @block.vector
        def _(dve):
            for n in range(NIT):
                j = n % 2
                dve.wait_ge(sa, 2 * n + 2)
                if n >= 2:
                    # wv[j] was read by matmul #(n-2) -> spe >= n-1
                    dve.wait_ge(spe, n - 1)
                dve.scalar_tensor_tensor(out=wv[j][:], in0=u[j][:], scalar=1.0,
                                         in1=e[j][:], op0=A.subtract,
                                         op1=A.mult).then_inc(sv)
            dve.wait_ge(sa, 2 * NIT + 1)
            dve.tensor_scalar_mul(mean[:], ysum[:], 1.0 / B).then_inc(sv)
            dve.wait_ge(sa, 2 * NIT + 2)
            dve.tensor_scalar(out=msq[:], in0=ssq[:], scalar1=1.0 / B,
                              scalar2=BN_EPS, op0=A.mult,
                              op1=A.add).then_inc(sv)
            dve.wait_ge(sv, NIT + 1)
            dve.tensor_mul(m2[:], mean[:], mean[:]).then_inc(sv)
            dve.wait_ge(sv, NIT + 3)
            dve.tensor_sub(var[:], msq[:], m2[:]).then_inc(sv)     # NIT+4
            dve.wait_ge(sa, 2 * NIT + 3)
            dve.reciprocal(rstd[:], std[:]).then_inc(sv)
            dve.wait_ge(sv, NIT + 5)
            dve.tensor_mul(ga[:], rstd[:], gamma_ap).then_inc(sv)
            dve.wait_ge(sv, NIT + 6)
            dve.tensor_mul(mga[:], mean[:], ga[:]).then_inc(sv)
            dve.wait_ge(sv, NIT + 7)
            dve.tensor_sub(bb[:], beta_ap, mga[:]).then_inc(sv)
            dve.wait_ge(sv, NIT + 8)
            dve.tensor_scalar(out=out_sb[:], in0=ysb[:], scalar1=ga[:],
                              scalar2=bb[:], op0=A.mult,
                              op1=A.add).then_inc(sv)              # NIT+9

        @block.tensor
        def _(pe):
            n = 0
            for o in range(OS):
                for k in range(NK):
                    col = k * OS + o
                    pe.wait_ge(sv, n + 1)
                    pe.matmul(psum[o:o + 1, :],
                              lhsT=big[:, G_WC0 + col:G_WC0 + col + 1],
                              rhs=wv[n % 2][:], start=(k == 0),
                              stop=(k == NK - 1)).then_inc(spe)
                    n += 1
    return nc


def _get_program(name: str) -> bass.Bass:
    if name not in _programs:
        _programs[name] = _build_fast() if name == "fast" else _build_general()
    return _programs[name]


def _pack_k(v2d: np.ndarray) -> np.ndarray:
    """(I, C) -> (KP, NK*C): out[p, k*C:(k+1)*C] = v2d[k*KP+p, :]."""
    c = v2d.shape[1]
    return np.ascontiguousarray(
        v2d.reshape(NK, KP, c).transpose(1, 0, 2).reshape(KP, NK * c))


def _pack_wc(w_shard, gamma_shard, beta_shard):
    wcm = np.zeros((KP, WCOLS), dtype=np.float32)
    wcm[:, :NK * OS] = _pack_k(w_shard)
    wcm[:OS, NK * OS] = gamma_shard
    wcm[:OS, NK * OS + 1] = beta_shard
    return wcm


_last_results = None  # BassKernelResults of the most recent run (for test.py)
TRACE = False
TRACE_KW: dict = {}


def _make_in_maps(x, scale, bias, weight, gamma, beta):
    """Returns (program_name, in_maps)."""
    fast = bool(np.all(scale == scale[:, :1]) and np.all(bias == bias[:, :1]))

    with np.errstate(divide="ignore", invalid="ignore"):
        inv_s = (1.0 / scale).astype(np.float32)
        nb_s = (-bias / scale).astype(np.float32)

    in_maps = []
    if fast:
        xpart = np.empty((KP, NK * XCOLS_F), dtype=np.float32)
        for k in range(NK):
            c0 = k * XCOLS_F
            ksl = slice(k * KP, (k + 1) * KP)
            xpart[:, c0] = inv_s[ksl, 0]
            xpart[:, c0 + 1] = nb_s[ksl, 0]
            xpart[:, c0 + 2:c0 + 2 + B] = x[:, ksl].T
        for c in range(N_CORES):
            osl = slice(c * OS, (c + 1) * OS)
            ab = np.concatenate(
                [xpart, _pack_wc(weight[:, osl], gamma[osl], beta[osl])],
                axis=1)
            in_maps.append({"ab": np.ascontiguousarray(ab)})
    else:
        xt_p = np.ascontiguousarray(
            x.T.reshape(NK, KP, B).transpose(1, 0, 2).reshape(KP, NK * B))
        for c in range(N_CORES):
            osl = slice(c * OS, (c + 1) * OS)
            ab = np.concatenate(
                [xt_p,
                 _pack_wc(weight[:, osl], gamma[osl], beta[osl]),
                 _pack_k(inv_s[:, osl]),
                 _pack_k(nb_s[:, osl])], axis=1)
            in_maps.append({"ab": np.ascontiguousarray(ab)})
    return ("fast" if fast else "general"), in_maps


def kernel(x, scale, bias, weight, gamma, beta):
    x = np.asarray(x, dtype=np.float32)
    scale = np.asarray(scale, dtype=np.float32)
    bias = np.asarray(bias, dtype=np.float32)
    # MEXHAT_NORM folded into the weights (device computes (t^2-1)e^{-t^2/2})
    weight = np.asarray(weight, dtype=np.float32) * np.float32(MEXHAT_NORM)
    gamma = np.asarray(gamma, dtype=np.float32)
    beta = np.asarray(beta, dtype=np.float32)
    assert x.shape == (B, I) and weight.shape == (I, O)

    which, in_maps = _make_in_maps(x, scale, bias, weight, gamma, beta)
    nc = _get_program(which)
    res = run_bass_kernel_spmd(nc, in_maps, list(range(N_CORES)),
                               trace=TRACE, **TRACE_KW)
    global _last_results
    _last_results = res

    out = np.empty((B, O), dtype=np.float32)
    for c in range(N_CORES):
        out[:, c * OS:(c + 1) * OS] = res.results[c]["yT"].T
    return out
